# revision 1
# baseline (speedup 1.0000x reference)
"""Trainium kernel for nn_Attention_62569083568830 (sparse_attention).

Strategy: data-parallel over batch B=32 across 8 NeuronCores (4 batches each).
All FFTs are expressed as small dense matmuls against precomputed real DFT
matrices (N=325 spatial, T=12 temporal), so the device graph is pure
matmul/elementwise/softmax work that maps onto the TensorE/VectorE/ScalarE
engines. Two exact algebraic simplifications make this cheap:

1. Temporal branch: softmax rows sum to 1, and the value tensor broadcasts
   along the softmax axis, so (vg * attw).mean(axis=-1) == vf / Mt exactly.
   The whole temporal attention collapses to a fixed [12,12] low-pass matrix
   applied along the temporal view axis of v = x @ Wv_t^T.
2. Global Frobenius norms ||q||, ||k|| (over the FULL unsharded tensors) are
   computed from the 64x64 Gram matrix G = X^T X via ||X W^T||_F^2 =
   tr(W G W^T). This is O(R*D^2) host prep on the raw input, removing the
   only cross-core dependency, so the 8 cores run fully independently.

Inputs are sharded/replicated on host, the math runs on the 8 axon-tunneled
NeuronCores via a single pmapped program, and the full [32,12,325,64] fp32
output is gathered back.
"""

import numpy as np

B, T, N, D = 32, 12, 325, 64
H = 4
HD = D // H
M_SP = 32
M_T = T // 2
SCALE = HD ** -0.5
NCORES = 8
BS = B // NCORES

_CACHE = {}


def _consts(sp_modes, t_modes, weights_Q):
    fm = np.asarray(sp_modes).astype(np.int64)
    n = np.arange(N)
    ang = 2.0 * np.pi * np.outer(n, fm) / N          # [N, M]
    Cre = np.cos(ang).astype(np.float32)             # rfft real part
    Cim = (-np.sin(ang)).astype(np.float32)          # rfft imag part
    cj = np.where(fm == 0, 1.0, 2.0)                 # irfft symmetry weight
    Gre = (cj[:, None] * np.cos(ang.T) / N).astype(np.float32)   # [M, N]
    Gim = (-cj[:, None] * np.sin(ang.T) / N).astype(np.float32)  # [M, N]
    # temporal low-pass matrix: irfft(keep t_modes(rfft(.))) / M_T along T
    mask = np.zeros(T // 2 + 1)
    mask[np.asarray(t_modes).astype(np.int64)] = 1.0
    eye = np.eye(T)
    Lmat = (np.fft.irfft(np.fft.rfft(eye, axis=0) * mask[:, None], n=T, axis=0)
            / M_T).astype(np.float32)                # [T, T], y = Lmat @ v
    Wabs = np.abs(np.asarray(weights_Q)).astype(np.float32)      # [M, M-1, HD]
    return Cre, Cim, Gre, Gim, Lmat, Wabs


def _shard_fn(xs, adj, Wq, Wk, Wv, Wvt, Wfc1, Wmlp, bmlp,
              Wabs, Cre, Cim, Gre, Gim, Lmat, inv_nq, inv_nk):
    import jax.numpy as jnp
    import jax
    Bs = xs.shape[0]
    # ---- GCN branch ----
    a = adj / jnp.sum(adj, axis=1, keepdims=True)
    agg = jnp.einsum('btkd,nk->btnd', xs, a)
    hmid = jnp.einsum('btnd,ed->btne', agg, Wfc1)
    gcn = jnp.einsum('btnd,ed->btne', hmid, Wmlp) + bmlp

    # ---- Spatial branch (frequency attention over nodes) ----
    q = jnp.einsum('btnd,ed->btne', xs, Wq)
    k = jnp.einsum('btnd,ed->btne', xs, Wk)
    v = jnp.einsum('btnd,ed->btne', xs, Wv)
    prep = lambda y: y.reshape(Bs, T, N, H, HD).transpose(0, 1, 3, 4, 2)
    qp, kp, vp = prep(q), prep(k), prep(v)           # [Bs,T,H,HD,N]
    qf_re = jnp.einsum('bthen,nm->bthem', qp, Cre)
    qf_im = jnp.einsum('bthen,nm->bthem', qp, Cim)
    kf_re = jnp.einsum('bthen,nm->bthem', kp, Cre)
    kf_im = jnp.einsum('bthen,nm->bthem', kp, Cim)
    vf_re = jnp.einsum('bthen,nm->bthem', vp, Cre)
    vf_im = jnp.einsum('bthen,nm->bthem', vp, Cim)
    Qabs = jnp.sqrt(qf_re ** 2 + qf_im ** 2) * inv_nq   # [b,t,h,e,m]
    Kabs = jnp.sqrt(kf_re ** 2 + kf_im ** 2) * inv_nk   # [b,t,h,e,j]
    # W'[m,j,e]: col j=0 is |qf|[e,m]; cols j>=1 are |weights_Q|[m,j-1,e]
    col0 = Qabs.transpose(0, 1, 2, 4, 3)[:, :, :, :, None, :]      # [b,t,h,m,1,e]
    rest = jnp.broadcast_to(Wabs[None, None, None],
                            (Bs, T, H, M_SP, M_SP - 1, HD))
    Wfull = jnp.concatenate([col0, rest], axis=4)                  # [b,t,h,m,j,e]
    Kfac = Kabs.transpose(0, 1, 2, 4, 3)[:, :, :, None, :, :]      # [b,t,h,1,j,e]
    z = SCALE * Kfac * Wfull
    attw = jax.nn.softmax(z, axis=4)
    S = jnp.mean(attw, axis=3)                                     # [b,t,h,j,e]
    St = S.transpose(0, 1, 2, 4, 3)                                # [b,t,h,e,j]
    o_re = vf_re * St
    o_im = vf_im * St
    ysp = (jnp.einsum('bthej,jn->bthen', o_re, Gre)
           + jnp.einsum('bthej,jn->bthen', o_im, Gim))             # [b,t,h,e,n]
    ysp = ysp.transpose(0, 1, 4, 2, 3).reshape(Bs, T, N, D)

    # ---- Temporal branch (collapses to low-pass on v) ----
    vt = jnp.einsum('btnd,ed->btne', xs, Wvt)
    vt_view = vt.reshape(Bs, N, T, H, HD)            # raw buffer reinterpret
    yt = jnp.einsum('st,bnthe->bnshe', Lmat, vt_view)
    yt = yt.transpose(0, 2, 1, 3, 4).reshape(Bs, T, N, D)

    return gcn + ysp + yt


def kernel(x, adj, Wq_geo, Wk_geo, Wv_geo, Wq_t, Wk_t, Wv_t,
           W_fc1, W_mlp, b_mlp, weights_Q, weights_Q_t, sp_modes, t_modes):
    x = np.asarray(x, dtype=np.float32)
    adj = np.asarray(adj, dtype=np.float32)
    Wq, Wk, Wv = (np.asarray(w, np.float32) for w in (Wq_geo, Wk_geo, Wv_geo))
    Wvt = np.asarray(Wv_t, np.float32)
    Wfc1, Wmlp, bmlp = (np.asarray(w, np.float32) for w in (W_fc1, W_mlp, b_mlp))

    Cre, Cim, Gre, Gim, Lmat, Wabs = _consts(sp_modes, t_modes, weights_Q)

    # Global Frobenius norms of q/k via the Gram-matrix identity (host prep;
    # removes the only cross-core reduction).
    X = x.reshape(-1, D).astype(np.float64)
    G = X.T @ X
    nq = float(np.sqrt(np.sum((Wq.astype(np.float64) @ G) * Wq)))
    nk = float(np.sqrt(np.sum((Wk.astype(np.float64) @ G) * Wk)))
    inv_nq = np.float32(1.0 / nq)
    inv_nk = np.float32(1.0 / nk)

    import jax
    try:
        devs = [d for d in jax.devices() if d.platform != 'cpu'][:NCORES]
        if len(devs) < NCORES:
            raise RuntimeError('need 8 neuron cores')
        if 'pm' not in _CACHE:
            _CACHE['pm'] = jax.pmap(
                _shard_fn,
                in_axes=(0,) + (None,) * 15,
                devices=devs)
        xs_all = x.reshape(NCORES, BS, T, N, D)
        out = _CACHE['pm'](xs_all, adj, Wq, Wk, Wv, Wvt, Wfc1, Wmlp, bmlp,
                           Wabs, Cre, Cim, Gre, Gim, Lmat, inv_nq, inv_nk)
        out = np.asarray(out).reshape(B, T, N, D)
    except Exception:
        # fallback: same math on host CPU
        with jax.default_device(jax.devices('cpu')[0]):
            out = np.asarray(_shard_fn(
                x, adj, Wq, Wk, Wv, Wvt, Wfc1, Wmlp, bmlp,
                Wabs, Cre, Cim, Gre, Gim, Lmat, inv_nq, inv_nk))
    return out.astype(np.float32)


# revision 2
# speedup vs baseline: 17.7977x; 17.7977x over previous
"""Trainium kernel for nn_Attention_62569083568830 (sparse_attention).

Strategy: data-parallel over batch B=32 across 8 NeuronCores (4 batches each).
All FFTs are expressed as small dense matmuls against precomputed real DFT
matrices (N=325 spatial, T=12 temporal), so the device graph is pure
matmul/elementwise/softmax work that maps onto the TensorE/VectorE/ScalarE
engines. Two exact algebraic simplifications make this cheap:

1. Temporal branch: softmax rows sum to 1, and the value tensor broadcasts
   along the softmax axis, so (vg * attw).mean(axis=-1) == vf / Mt exactly.
   The whole temporal attention collapses to a fixed [12,12] low-pass matrix
   applied along the temporal view axis of v = x @ Wv_t^T.
2. Global Frobenius norms ||q||, ||k|| (over the FULL unsharded tensors) are
   computed from the 64x64 Gram matrix G = X^T X via ||X W^T||_F^2 =
   tr(W G W^T). This is O(R*D^2) host prep on the raw input, removing the
   only cross-core dependency, so the 8 cores run fully independently.

Inputs are sharded/replicated on host, the math runs on the 8 axon-tunneled
NeuronCores via a single pmapped program, and the full [32,12,325,64] fp32
output is gathered back.
"""

import numpy as np

B, T, N, D = 32, 12, 325, 64
H = 4
HD = D // H
M_SP = 32
M_T = T // 2
SCALE = HD ** -0.5
NCORES = 8
BS = B // NCORES

_CACHE = {}


def _consts(sp_modes, t_modes, weights_Q):
    fm = np.asarray(sp_modes).astype(np.int64)
    n = np.arange(N)
    ang = 2.0 * np.pi * np.outer(n, fm) / N          # [N, M]
    Cre = np.cos(ang).astype(np.float32)             # rfft real part
    Cim = (-np.sin(ang)).astype(np.float32)          # rfft imag part
    cj = np.where(fm == 0, 1.0, 2.0)                 # irfft symmetry weight
    Gre = (cj[:, None] * np.cos(ang.T) / N).astype(np.float32)   # [M, N]
    Gim = (-cj[:, None] * np.sin(ang.T) / N).astype(np.float32)  # [M, N]
    # temporal low-pass matrix: irfft(keep t_modes(rfft(.))) / M_T along T
    mask = np.zeros(T // 2 + 1)
    mask[np.asarray(t_modes).astype(np.int64)] = 1.0
    eye = np.eye(T)
    Lmat = (np.fft.irfft(np.fft.rfft(eye, axis=0) * mask[:, None], n=T, axis=0)
            / M_T).astype(np.float32)                # [T, T], y = Lmat @ v
    Wabs = np.abs(np.asarray(weights_Q)).astype(np.float32)      # [M, M-1, HD]
    return Cre, Cim, Gre, Gim, Lmat, Wabs


def _shard_fn(xs, adj, Wq, Wk, Wv, Wvt, Wfc1, Wmlp, bmlp,
              Wabs, Cre, Cim, Gre, Gim, Lmat, inv_nq, inv_nk):
    import jax.numpy as jnp
    import jax
    Bs = xs.shape[0]
    # ---- GCN branch ----
    a = adj / jnp.sum(adj, axis=1, keepdims=True)
    agg = jnp.einsum('btkd,nk->btnd', xs, a)
    hmid = jnp.einsum('btnd,ed->btne', agg, Wfc1)
    gcn = jnp.einsum('btnd,ed->btne', hmid, Wmlp) + bmlp

    # ---- Spatial branch (frequency attention over nodes) ----
    q = jnp.einsum('btnd,ed->btne', xs, Wq)
    k = jnp.einsum('btnd,ed->btne', xs, Wk)
    v = jnp.einsum('btnd,ed->btne', xs, Wv)
    prep = lambda y: y.reshape(Bs, T, N, H, HD).transpose(0, 1, 3, 4, 2)
    qp, kp, vp = prep(q), prep(k), prep(v)           # [Bs,T,H,HD,N]
    qf_re = jnp.einsum('bthen,nm->bthem', qp, Cre)
    qf_im = jnp.einsum('bthen,nm->bthem', qp, Cim)
    kf_re = jnp.einsum('bthen,nm->bthem', kp, Cre)
    kf_im = jnp.einsum('bthen,nm->bthem', kp, Cim)
    vf_re = jnp.einsum('bthen,nm->bthem', vp, Cre)
    vf_im = jnp.einsum('bthen,nm->bthem', vp, Cim)
    Qabs = jnp.sqrt(qf_re ** 2 + qf_im ** 2) * inv_nq   # [b,t,h,e,m]
    Kabs = jnp.sqrt(kf_re ** 2 + kf_im ** 2) * inv_nk   # [b,t,h,e,j]
    # W'[m,j,e]: col j=0 is |qf|[e,m]; cols j>=1 are |weights_Q|[m,j-1,e]
    col0 = Qabs.transpose(0, 1, 2, 4, 3)[:, :, :, :, None, :]      # [b,t,h,m,1,e]
    rest = jnp.broadcast_to(Wabs[None, None, None],
                            (Bs, T, H, M_SP, M_SP - 1, HD))
    Wfull = jnp.concatenate([col0, rest], axis=4)                  # [b,t,h,m,j,e]
    Kfac = Kabs.transpose(0, 1, 2, 4, 3)[:, :, :, None, :, :]      # [b,t,h,1,j,e]
    z = SCALE * Kfac * Wfull
    attw = jax.nn.softmax(z, axis=4)
    S = jnp.mean(attw, axis=3)                                     # [b,t,h,j,e]
    St = S.transpose(0, 1, 2, 4, 3)                                # [b,t,h,e,j]
    o_re = vf_re * St
    o_im = vf_im * St
    ysp = (jnp.einsum('bthej,jn->bthen', o_re, Gre)
           + jnp.einsum('bthej,jn->bthen', o_im, Gim))             # [b,t,h,e,n]
    ysp = ysp.transpose(0, 1, 4, 2, 3).reshape(Bs, T, N, D)

    # ---- Temporal branch (collapses to low-pass on v) ----
    vt = jnp.einsum('btnd,ed->btne', xs, Wvt)
    vt_view = vt.reshape(Bs, N, T, H, HD)            # raw buffer reinterpret
    yt = jnp.einsum('st,bnthe->bnshe', Lmat, vt_view)
    yt = yt.transpose(0, 2, 1, 3, 4).reshape(Bs, T, N, D)

    return gcn + ysp + yt


def kernel(x, adj, Wq_geo, Wk_geo, Wv_geo, Wq_t, Wk_t, Wv_t,
           W_fc1, W_mlp, b_mlp, weights_Q, weights_Q_t, sp_modes, t_modes):
    x = np.asarray(x, dtype=np.float32)
    adj = np.asarray(adj, dtype=np.float32)
    Wq, Wk, Wv = (np.asarray(w, np.float32) for w in (Wq_geo, Wk_geo, Wv_geo))
    Wvt = np.asarray(Wv_t, np.float32)
    Wfc1, Wmlp, bmlp = (np.asarray(w, np.float32) for w in (W_fc1, W_mlp, b_mlp))

    Cre, Cim, Gre, Gim, Lmat, Wabs = _consts(sp_modes, t_modes, weights_Q)

    # Global Frobenius norms of q/k via the Gram-matrix identity (host prep;
    # removes the only cross-core reduction).
    X = x.reshape(-1, D).astype(np.float64)
    G = X.T @ X
    nq = float(np.sqrt(np.sum((Wq.astype(np.float64) @ G) * Wq)))
    nk = float(np.sqrt(np.sum((Wk.astype(np.float64) @ G) * Wk)))
    inv_nq = np.float32(1.0 / nq)
    inv_nk = np.float32(1.0 / nk)

    import jax
    try:
        devs = [d for d in jax.devices() if d.platform != 'cpu'][:NCORES]
        if len(devs) < NCORES:
            raise RuntimeError('need 8 neuron cores')
        if 'pm' not in _CACHE:
            _CACHE['pm'] = jax.pmap(
                _shard_fn,
                in_axes=(0,) + (None,) * 16,
                devices=devs)
        xs_all = x.reshape(NCORES, BS, T, N, D)
        out = _CACHE['pm'](xs_all, adj, Wq, Wk, Wv, Wvt, Wfc1, Wmlp, bmlp,
                           Wabs, Cre, Cim, Gre, Gim, Lmat, inv_nq, inv_nk)
        out = np.asarray(out).reshape(B, T, N, D)
    except Exception:
        # fallback: same math on host CPU
        with jax.default_device(jax.devices('cpu')[0]):
            out = np.asarray(_shard_fn(
                x, adj, Wq, Wk, Wv, Wvt, Wfc1, Wmlp, bmlp,
                Wabs, Cre, Cim, Gre, Gim, Lmat, inv_nq, inv_nk))
    return out.astype(np.float32)
